# Initial kernel scaffold
#
"""Trainium2 Bass kernel for nn_EnigmaAttention: causal RoPE attention.

Sharding: tensor-parallel over heads (16 heads / 8 cores = 2 heads per core).
Each core:
  1. projects q/k/v for its 2 heads in transposed layout (channels on
     partitions) directly off a host-pretransposed xT,
  2. applies RoPE (partition-swap via SBUF-SBUF DMA + sign-baked sin table),
  3. runs block-causal attention in score-transposed orientation
     (softmax without max-subtraction — scores are bounded by |q||k|/sqrt(hd);
     row sums come from a ones-vector matmul on the PE),
  4. AllGathers the per-head attention outputs (channel-major) across cores,
  5. computes its 256-column slice of the output projection.
Host side only slices/transposes/casts inputs and concatenates outputs.
"""
import sys

sys.path.insert(0, "/opt/trn_rl_repo")

import numpy as np
import ml_dtypes

import concourse.bass as bass
import concourse.mybir as mybir
import concourse.tile as tile
from concourse.bass_utils import run_bass_kernel_spmd

BF16 = mybir.dt.bfloat16
F32 = mybir.dt.float32
AF = mybir.ActivationFunctionType

N_CORES = 8
B, S, D = 2, 2048, 2048
H = 16
HD = D // H            # 128 head dim
HL = H // N_CORES      # 2 local heads
CH = HL * HD           # 256 local qkv channels
OC = D // N_CORES      # 256 local output columns
TCH = 512              # token chunk
KB = 128               # k block
QB = 512               # q chunk
ROPE_BASE = 10000.0


def build_program(seq=S, batch=B):
    s, t = seq, batch * seq
    ntch = t // TCH
    nkb = s // 128            # k blocks per batch row
    scale = float(HD) ** -0.5

    nc = bass.Bass(num_devices=N_CORES)
    xT = nc.declare_dram_parameter("xT", [D, t], BF16, isOutput=False)
    wqT = nc.declare_dram_parameter("wqT", [D, CH], BF16, isOutput=False)
    wkT = nc.declare_dram_parameter("wkT", [D, CH], BF16, isOutput=False)
    wvT = nc.declare_dram_parameter("wvT", [D, CH], BF16, isOutput=False)
    woT = nc.declare_dram_parameter("woT", [D, OC], BF16, isOutput=False)
    cosT = nc.declare_dram_parameter("cosT", [HD, s], F32, isOutput=False)
    sinS = nc.declare_dram_parameter("sinS", [HD, s], F32, isOutput=False)
    outT = nc.declare_dram_parameter("outT", [OC, t], F32, isOutput=True)

    ag_out = nc.dram_tensor("ag_out", [N_CORES * CH, t], BF16, addr_space="Shared")

    nd = D // 128  # 16 contraction blocks

    with tile.TileContext(nc) as tc:
        with (
            tc.tile_pool(name="persist", bufs=1) as persist,
            tc.tile_pool(name="dram", bufs=1, space="DRAM") as dram,
        ):
            qTb = [persist.tile([HD, t], BF16, tag=f"qTb{h}") for h in range(HL)]
            kTb = [persist.tile([HD, t], BF16, tag=f"kTb{h}") for h in range(HL)]
            oTb = [persist.tile([HD, t], BF16, tag=f"oTb{h}") for h in range(HL)]
            v_sb = persist.tile([128, t // 128, CH], BF16, tag="v")
            wo_sb = persist.tile([128, nd, OC], BF16, tag="wo")
            nc.sync.dma_start(
                out=wo_sb[:], in_=woT.rearrange("(a p) c -> p a c", p=128)
            )
            agin = dram.tile([HL, HD, t], BF16)

            # ---------------- projections + rope ----------------
            with (
                tc.tile_pool(name="w3", bufs=1) as w3,
                tc.tile_pool(name="xin", bufs=2) as xin,
                tc.tile_pool(name="trig", bufs=1) as trig,
                tc.tile_pool(name="rope", bufs=3) as rope,
                tc.tile_pool(name="ppqk", bufs=3, space="PSUM") as ppqk,
                tc.tile_pool(name="ppv", bufs=2, space="PSUM") as ppv,
            ):
                wsbs = {}
                for name, wdram in (("q", wqT), ("k", wkT), ("v", wvT)):
                    wsb = w3.tile([128, nd, CH], BF16, tag=f"w{name}")
                    nc.sync.dma_start(
                        out=wsb[:], in_=wdram.rearrange("(a p) c -> p a c", p=128)
                    )
                    wsbs[name] = wsb
                cos_sb = trig.tile([HD, s], F32, tag="cos")
                nc.sync.dma_start(out=cos_sb[:], in_=cosT[:, :])
                sin_sb = trig.tile([HD, s], F32, tag="sin")
                nc.sync.dma_start(out=sin_sb[:], in_=sinS[:, :])

                for tch in range(ntch):
                    xc = xin.tile([128, nd, TCH], BF16, tag="xc")
                    nc.sync.dma_start(
                        out=xc[:],
                        in_=xT.rearrange("(a p) t -> p a t", p=128)[
                            :, :, bass.ts(tch, TCH)
                        ],
                    )
                    s0 = (tch * TCH) % s
                    for h in range(HL):
                        for wname, dest in (("q", qTb), ("k", kTb)):
                            ps = ppqk.tile([128, TCH], F32, tag="qk")
                            for dblk in range(nd):
                                nc.tensor.matmul(
                                    ps[:],
                                    lhsT=wsbs[wname][:, dblk, h * HD : (h + 1) * HD],
                                    rhs=xc[:, dblk, :],
                                    start=(dblk == 0),
                                    stop=(dblk == nd - 1),
                                )
                            zf = rope.tile([128, TCH], F32, tag="zf")
                            nc.scalar.activation(zf[:], ps[:], AF.Copy)
                            zs = rope.tile([128, TCH], F32, tag="zs")
                            nc.sync.dma_start(out=zs[0:64, :], in_=zf[64:128, :])
                            nc.sync.dma_start(out=zs[64:128, :], in_=zf[0:64, :])
                            t1 = rope.tile([128, TCH], F32, tag="t1")
                            nc.vector.tensor_mul(
                                t1[:], zf[:], cos_sb[:, s0 : s0 + TCH]
                            )
                            t2 = rope.tile([128, TCH], F32, tag="t2")
                            nc.vector.tensor_mul(
                                t2[:], zs[:], sin_sb[:, s0 : s0 + TCH]
                            )
                            nc.vector.tensor_add(
                                dest[h][:, tch * TCH : (tch + 1) * TCH], t1[:], t2[:]
                            )
                    for tp in range(TCH // 128):
                        vps = ppv.tile([128, CH], F32, tag="vp")
                        for dblk in range(nd):
                            nc.tensor.matmul(
                                vps[:],
                                lhsT=xc[:, dblk, tp * 128 : (tp + 1) * 128],
                                rhs=wsbs["v"][:, dblk, :],
                                start=(dblk == 0),
                                stop=(dblk == nd - 1),
                            )
                        nc.scalar.activation(
                            v_sb[:, tch * (TCH // 128) + tp, :], vps[:], AF.Copy
                        )

            # ---------------- attention ----------------
            with (
                tc.tile_pool(name="att", bufs=4) as att,
                tc.tile_pool(name="onesp", bufs=1) as onesp,
                tc.tile_pool(name="pst", bufs=2, space="PSUM") as pst,
                tc.tile_pool(name="pot", bufs=2, space="PSUM") as pot,
                tc.tile_pool(name="psm", bufs=2, space="PSUM") as psm,
                tc.tile_pool(name="pbc", bufs=2, space="PSUM") as pbc,
            ):
                ones_col = onesp.tile([128, 1], BF16, tag="onec")
                nc.vector.memset(ones_col[:], 1.0)
                ones_row = onesp.tile([1, 128], F32, tag="oner")
                nc.vector.memset(ones_row[:], 1.0)
                for h in range(HL):
                    for b in range(batch):
                        t0 = b * s
                        for qc in range(s // QB):
                            ot = pot.tile([128, QB], F32, tag="ot")
                            sm = psm.tile([1, QB], F32, tag="sm")
                            nkj = (qc + 1) * (QB // KB)
                            for kj in range(nkj):
                                qoff = max(0, kj * KB - qc * QB)
                                width = QB - qoff
                                st = pst.tile([128, QB], F32, tag="st")
                                nc.tensor.matmul(
                                    st[:, :width],
                                    lhsT=kTb[h][:, t0 + kj * KB : t0 + (kj + 1) * KB],
                                    rhs=qTb[h][
                                        :, t0 + qc * QB + qoff : t0 + (qc + 1) * QB
                                    ],
                                    start=True,
                                    stop=True,
                                )
                                e = att.tile([128, QB], BF16, tag="e")
                                nc.scalar.activation(
                                    e[:, :width], st[:, :width], AF.Exp, scale=scale
                                )
                                if kj * KB >= qc * QB:
                                    # diagonal block: zero strictly-below-diag
                                    nc.gpsimd.affine_select(
                                        out=e[:, :KB],
                                        in_=e[:, :KB],
                                        pattern=[[1, KB]],
                                        compare_op=mybir.AluOpType.is_ge,
                                        fill=0.0,
                                        base=0,
                                        channel_multiplier=-1,
                                    )
                                nc.tensor.matmul(
                                    ot[:, qoff:],
                                    lhsT=v_sb[
                                        :, b * nkb + kj, h * HD : (h + 1) * HD
                                    ],
                                    rhs=e[:, :width],
                                    start=(kj == 0),
                                    stop=(kj == nkj - 1),
                                )
                                nc.tensor.matmul(
                                    sm[:, qoff:],
                                    lhsT=ones_col[:],
                                    rhs=e[:, :width],
                                    start=(kj == 0),
                                    stop=(kj == nkj - 1),
                                )
                            rec = att.tile([1, QB], F32, tag="rec")
                            nc.vector.reciprocal(rec[:], sm[:])
                            bc = pbc.tile([128, QB], F32, tag="bc")
                            nc.tensor.matmul(
                                bc[:], lhsT=ones_row[:], rhs=rec[:],
                                start=True, stop=True,
                            )
                            nc.vector.tensor_mul(
                                oTb[h][:, t0 + qc * QB : t0 + (qc + 1) * QB],
                                ot[:],
                                bc[:],
                            )
                for h in range(HL):
                    nc.sync.dma_start(out=agin[h], in_=oTb[h][:])
                nc.gpsimd.collective_compute(
                    "AllGather",
                    mybir.AluOpType.bypass,
                    replica_groups=[list(range(N_CORES))],
                    ins=[agin[:]],
                    outs=[ag_out[:]],
                )

            # ---------------- output projection slice ----------------
            with (
                tc.tile_pool(name="ocin", bufs=2) as ocin,
                tc.tile_pool(name="osout", bufs=3) as osout,
                tc.tile_pool(name="pout", bufs=2, space="PSUM") as pout,
            ):
                for tch in range(ntch):
                    occ = ocin.tile([128, nd, TCH], BF16, tag="occ")
                    nc.sync.dma_start(
                        out=occ[:],
                        in_=ag_out.rearrange("(a p) t -> p a t", p=128)[
                            :, :, bass.ts(tch, TCH)
                        ],
                    )
                    for ob in range(OC // 128):
                        ps = pout.tile([128, TCH], F32, tag="po")
                        for ib in range(nd):
                            nc.tensor.matmul(
                                ps[:],
                                lhsT=wo_sb[:, ib, ob * 128 : (ob + 1) * 128],
                                rhs=occ[:, ib, :],
                                start=(ib == 0),
                                stop=(ib == nd - 1),
                            )
                        osb = osout.tile([128, TCH], F32, tag="osb")
                        nc.scalar.activation(osb[:], ps[:], AF.Copy)
                        nc.sync.dma_start(
                            out=outT[
                                ob * 128 : (ob + 1) * 128,
                                tch * TCH : (tch + 1) * TCH,
                            ],
                            in_=osb[:],
                        )
    return nc


def host_inputs(x, Wq, Wk, Wv, Wo, seq=S, batch=B):
    """Slice/transpose/cast the full inputs into per-core input maps."""
    bf = ml_dtypes.bfloat16
    t = batch * seq
    x = np.asarray(x, dtype=np.float32)
    xTb = np.ascontiguousarray(x.reshape(t, D).T).astype(bf)
    inv_freq = 1.0 / (
        ROPE_BASE ** (np.arange(0, HD, 2, dtype=np.float32) / HD)
    )
    pos = np.arange(seq, dtype=np.float32)
    freqs = np.einsum("i,j->ij", pos, inv_freq)
    emb = np.concatenate([freqs, freqs], axis=-1)            # [s, HD]
    cosT_np = np.ascontiguousarray(np.cos(emb).T).astype(np.float32)
    sinT = np.sin(emb).T.astype(np.float32)
    sinS_np = np.ascontiguousarray(
        np.concatenate([-sinT[: HD // 2], sinT[HD // 2 :]], axis=0)
    )
    in_maps = []
    for c in range(N_CORES):
        sl = slice(c * CH, (c + 1) * CH)
        in_maps.append(
            {
                "xT": xTb,
                "wqT": np.ascontiguousarray(np.asarray(Wq)[sl].T).astype(bf),
                "wkT": np.ascontiguousarray(np.asarray(Wk)[sl].T).astype(bf),
                "wvT": np.ascontiguousarray(np.asarray(Wv)[sl].T).astype(bf),
                "woT": np.ascontiguousarray(np.asarray(Wo)[sl].T).astype(bf),
                "cosT": cosT_np,
                "sinS": sinS_np,
            }
        )
    return in_maps


_PROGRAM = None


def kernel(x, Wq, Wk, Wv, Wo):
    global _PROGRAM
    if _PROGRAM is None:
        _PROGRAM = build_program()
    in_maps = host_inputs(x, Wq, Wk, Wv, Wo)
    res = run_bass_kernel_spmd(_PROGRAM, in_maps, list(range(N_CORES)))
    outT_full = np.concatenate(
        [res.results[c]["outT"] for c in range(N_CORES)], axis=0
    )  # [D, T]
    return np.ascontiguousarray(outT_full.T).reshape(B, S, D).astype(np.float32)


if __name__ == "__main__":
    xs = np.random.randn(B, S, D).astype(np.float32)
    ws = [
        (np.random.randn(D, D) * D**-0.5).astype(np.float32) for _ in range(4)
    ]
    out = kernel(xs, *ws)
    print(out.shape, out.dtype)


# revision 9
# speedup vs baseline: 1.1317x; 1.1317x over previous
"""Trainium2 Bass kernel for nn_EnigmaAttention: causal RoPE attention.

Sharding: tensor-parallel over heads (16 heads / 8 cores = 2 heads per core).
Each core:
  1. projects q/k/v for its 2 heads in transposed layout (channels on
     partitions) directly off a host-pretransposed xT,
  2. applies RoPE (partition-swap via PSUM->SBUF DMA + sign-baked sin table),
  3. runs block-causal attention in score-transposed orientation
     (softmax without max-subtraction — scores are bounded by |q||k|/sqrt(hd);
     row sums come from a ones-vector matmul on the PE),
  4. AllGathers the per-(head,batch) attention outputs across cores as four
     pieces that overlap with the remaining attention work,
  5. computes its 256-column slice of the output projection.
Host side only slices/transposes/casts inputs and concatenates outputs.
"""
import sys

sys.path.insert(0, "/opt/trn_rl_repo")

import numpy as np
import ml_dtypes

import concourse.bass as bass
import concourse.bacc as bacc
import concourse.mybir as mybir
import concourse.tile as tile
from concourse.bass_utils import run_bass_kernel_spmd

BF16 = mybir.dt.bfloat16
F32 = mybir.dt.float32
AF = mybir.ActivationFunctionType

N_CORES = 8
B, S, D = 2, 2048, 2048
H = 16
HD = D // H            # 128 head dim
HL = H // N_CORES      # 2 local heads
CH = HL * HD           # 256 local qkv channels
OC = D // N_CORES      # 256 local output columns
TCH = 512              # token chunk
KB = 128               # k block
QB = 512               # q chunk
ROPE_BASE = 10000.0


def build_program(seq=S, batch=B):
    s = seq
    t = batch * seq
    ntch = t // TCH
    nkb = s // 128            # k blocks per batch row
    scale = float(HD) ** -0.5
    nd = D // 128             # 16 contraction blocks

    nc = bacc.Bacc(num_devices=N_CORES)
    xT = nc.declare_dram_parameter("xT", [D, t], BF16, isOutput=False)
    wqT = nc.declare_dram_parameter("wqT", [D, CH], BF16, isOutput=False)
    wkT = nc.declare_dram_parameter("wkT", [D, CH], BF16, isOutput=False)
    wvT = nc.declare_dram_parameter("wvT", [D, CH], BF16, isOutput=False)
    woT = nc.declare_dram_parameter("woT", [D, OC], BF16, isOutput=False)
    cosT = nc.declare_dram_parameter("cosT", [HD, s], F32, isOutput=False)
    sinS = nc.declare_dram_parameter("sinS", [HD, s], F32, isOutput=False)
    outT = nc.declare_dram_parameter("outT", [OC, t], F32, isOutput=True)

    # per-(local-head, batch) AllGather outputs; rows = rank-major head dim
    agout = [
        [
            nc.dram_tensor(
                f"agout{h}{b}", [N_CORES * HD, s], BF16, addr_space="Shared"
            )
            for b in range(batch)
        ]
        for h in range(HL)
    ]

    with tile.TileContext(nc) as tc:
        with (
            tc.tile_pool(name="persist", bufs=1) as persist,
            tc.tile_pool(name="dram", bufs=1, space="DRAM") as dram,
        ):
            def hb_tiles(prefix):
                return [
                    [
                        persist.tile(
                            [HD, s], BF16,
                            tag=f"{prefix}{h}{b}", name=f"{prefix}{h}{b}",
                        )
                        for b in range(batch)
                    ]
                    for h in range(HL)
                ]

            qTb = hb_tiles("qTb")
            kTb = hb_tiles("kTb")
            oTb = hb_tiles("oTb")
            v_sb = [
                persist.tile([128, nkb, CH], BF16, tag=f"v{b}", name=f"v{b}")
                for b in range(batch)
            ]
            wo_sb = persist.tile([128, nd, OC], BF16, tag="wo")
            nc.sync.dma_start(
                out=wo_sb[:], in_=woT.rearrange("(a p) c -> p a c", p=128)
            )
            agin = [
                [
                    dram.tile([HD, s], BF16, name=f"agin{h}{b}")
                    for b in range(batch)
                ]
                for h in range(HL)
            ]

            # ---------------- projections + rope ----------------
            with (
                tc.tile_pool(name="w3", bufs=1) as w3,
                tc.tile_pool(name="xin", bufs=2) as xin,
                tc.tile_pool(name="trig", bufs=1) as trig,
                tc.tile_pool(name="rope", bufs=3) as rope,
                tc.tile_pool(name="ppqk", bufs=3, space="PSUM") as ppqk,
                tc.tile_pool(name="ppv", bufs=2, space="PSUM") as ppv,
            ):
                wsbs = {}
                for name, wdram in (("q", wqT), ("k", wkT), ("v", wvT)):
                    wsb = w3.tile([128, nd, CH], BF16, tag=f"w{name}", name=f"w{name}")
                    nc.sync.dma_start(
                        out=wsb[:], in_=wdram.rearrange("(a p) c -> p a c", p=128)
                    )
                    wsbs[name] = wsb
                cos_sb = trig.tile([HD, s], F32, tag="cos")
                nc.sync.dma_start(out=cos_sb[:], in_=cosT[:, :])
                sin_sb = trig.tile([HD, s], F32, tag="sin")
                nc.sync.dma_start(out=sin_sb[:], in_=sinS[:, :])

                for tch in range(ntch):
                    b = (tch * TCH) // s
                    sc = tch * TCH - b * s     # column offset within batch
                    xc = xin.tile([128, nd, TCH], BF16, tag="xc")
                    nc.sync.dma_start(
                        out=xc[:],
                        in_=xT.rearrange("(a p) t -> p a t", p=128)[
                            :, :, bass.ts(tch, TCH)
                        ],
                    )
                    for h in range(HL):
                        for wname, dest in (("q", qTb), ("k", kTb)):
                            ps = ppqk.tile([128, TCH], F32, tag="qk")
                            for dblk in range(nd):
                                nc.tensor.matmul(
                                    ps[:],
                                    lhsT=wsbs[wname][:, dblk, h * HD : (h + 1) * HD],
                                    rhs=xc[:, dblk, :],
                                    start=(dblk == 0),
                                    stop=(dblk == nd - 1),
                                )
                            zf = rope.tile([128, TCH], F32, tag="zf")
                            nc.scalar.activation(zf[:], ps[:], AF.Copy)
                            zs = rope.tile([128, TCH], F32, tag="zs")
                            nc.sync.dma_start(out=zs[0:64, :], in_=zf[64:128, :])
                            nc.sync.dma_start(out=zs[64:128, :], in_=zf[0:64, :])
                            t1 = rope.tile([128, TCH], F32, tag="t1")
                            nc.vector.tensor_mul(
                                t1[:], zf[:], cos_sb[:, sc : sc + TCH]
                            )
                            t2 = rope.tile([128, TCH], F32, tag="t2")
                            nc.vector.tensor_mul(
                                t2[:], zs[:], sin_sb[:, sc : sc + TCH]
                            )
                            nc.vector.tensor_add(
                                dest[h][b][:, sc : sc + TCH], t1[:], t2[:]
                            )
                    for tp in range(TCH // 128):
                        vps = ppv.tile([128, CH], F32, tag="vp")
                        for dblk in range(nd):
                            nc.tensor.matmul(
                                vps[:],
                                lhsT=xc[:, dblk, tp * 128 : (tp + 1) * 128],
                                rhs=wsbs["v"][:, dblk, :],
                                start=(dblk == 0),
                                stop=(dblk == nd - 1),
                            )
                        nc.scalar.activation(
                            v_sb[b][:, sc // 128 + tp, :], vps[:], AF.Copy
                        )

            # -------- attention (+ overlapped AllGathers) + o_proj --------
            LAG = 3
            with (
                tc.tile_pool(name="att", bufs=6) as att,
                tc.tile_pool(name="onesp", bufs=1) as onesp,
                tc.tile_pool(name="ocin", bufs=2) as ocin,
                tc.tile_pool(name="osout", bufs=3) as osout,
                tc.tile_pool(name="pst", bufs=3, space="PSUM") as pst,
                tc.tile_pool(name="pot", bufs=2, space="PSUM") as pot,
                tc.tile_pool(name="psm", bufs=1, space="PSUM") as psm,
                tc.tile_pool(name="pbc", bufs=1, space="PSUM") as pbc,
                tc.tile_pool(name="pout", bufs=1, space="PSUM") as pout,
            ):
                ones_col = onesp.tile([128, 1], BF16, tag="onec")
                nc.vector.memset(ones_col[:], 1.0)
                ones_row = onesp.tile([1, 128], F32, tag="oner")
                nc.vector.memset(ones_row[:], 1.0)
                for b in range(batch):
                    for h in range(HL):
                        for qc in range(s // QB):
                            ot = pot.tile([128, QB], F32, tag="ot")
                            sm = psm.tile([1, QB], F32, tag="sm")
                            nkj = (qc + 1) * (QB // KB)

                            pend = []

                            def flush(one):
                                kj, e, width, qoff = one
                                nc.tensor.matmul(
                                    ot[:, qoff:],
                                    lhsT=v_sb[b][:, kj, h * HD : (h + 1) * HD],
                                    rhs=e[:, :width],
                                    start=(kj == 0),
                                    stop=(kj == nkj - 1),
                                )
                                nc.tensor.matmul(
                                    sm[:, qoff:],
                                    lhsT=ones_col[:],
                                    rhs=e[:, :width],
                                    start=(kj == 0),
                                    stop=(kj == nkj - 1),
                                )

                            for kj in range(nkj):
                                qoff = max(0, kj * KB - qc * QB)
                                width = QB - qoff
                                st = pst.tile([128, QB], F32, tag="st")
                                nc.tensor.matmul(
                                    st[:, :width],
                                    lhsT=kTb[h][b][:, kj * KB : (kj + 1) * KB],
                                    rhs=qTb[h][b][
                                        :, qc * QB + qoff : (qc + 1) * QB
                                    ],
                                    start=True,
                                    stop=True,
                                )
                                e = att.tile([128, QB], BF16, tag="e")
                                nc.scalar.activation(
                                    e[:, :width], st[:, :width], AF.Exp, scale=scale
                                )
                                if kj * KB >= qc * QB:
                                    # diagonal block: zero where k > q
                                    nc.gpsimd.affine_select(
                                        out=e[:, :KB],
                                        in_=e[:, :KB],
                                        pattern=[[1, KB]],
                                        compare_op=mybir.AluOpType.is_ge,
                                        fill=0.0,
                                        base=0,
                                        channel_multiplier=-1,
                                    )
                                pend.append((kj, e, width, qoff))
                                if len(pend) > LAG:
                                    flush(pend.pop(0))
                            for one in pend:
                                flush(one)
                            rec = att.tile([1, QB], F32, tag="rec")
                            nc.vector.reciprocal(rec[:], sm[:])
                            bc = pbc.tile([128, QB], F32, tag="bc")
                            nc.tensor.matmul(
                                bc[:], lhsT=ones_row[:], rhs=rec[:],
                                start=True, stop=True,
                            )
                            bcs = att.tile([128, QB], F32, tag="bcs")
                            nc.scalar.activation(bcs[:], bc[:], AF.Copy)
                            nc.vector.tensor_mul(
                                oTb[h][b][:, qc * QB : (qc + 1) * QB],
                                ot[:],
                                bcs[:],
                            )
                        # ship this (head, batch) piece while the rest computes
                        nc.sync.dma_start(out=agin[h][b][:], in_=oTb[h][b][:])
                        nc.gpsimd.collective_compute(
                            "AllGather",
                            mybir.AluOpType.bypass,
                            replica_groups=[list(range(N_CORES))],
                            ins=[agin[h][b][:]],
                            outs=[agout[h][b][:]],
                        )

                # ---------------- output projection slice ----------------
                for tch in range(ntch):
                    b = (tch * TCH) // s
                    sc = tch * TCH - b * s
                    occ = ocin.tile([128, nd, TCH], BF16, tag="occ")
                    for hl in range(HL):
                        nc.sync.dma_start(
                            out=occ[:, hl * N_CORES : (hl + 1) * N_CORES, :],
                            in_=agout[hl][b].rearrange("(a p) t -> p a t", p=128)[
                                :, :, sc : sc + TCH
                            ],
                        )
                    for ob in range(OC // 128):
                        ps = pout.tile([128, TCH], F32, tag="po")
                        for j in range(nd):
                            # occ block j: hl = j // 8, rank = j % 8
                            # -> global head index for the weight blocks
                            gh = 2 * (j % N_CORES) + (j // N_CORES)
                            nc.tensor.matmul(
                                ps[:],
                                lhsT=wo_sb[:, gh, ob * 128 : (ob + 1) * 128],
                                rhs=occ[:, j, :],
                                start=(j == 0),
                                stop=(j == nd - 1),
                            )
                        osb = osout.tile([128, TCH], F32, tag="osb")
                        nc.scalar.activation(osb[:], ps[:], AF.Copy)
                        nc.sync.dma_start(
                            out=outT[
                                ob * 128 : (ob + 1) * 128,
                                tch * TCH : (tch + 1) * TCH,
                            ],
                            in_=osb[:],
                        )
    nc.finalize()
    return nc


def host_inputs(x, Wq, Wk, Wv, Wo, seq=S, batch=B):
    """Slice/transpose/cast the full inputs into per-core input maps."""
    bf = ml_dtypes.bfloat16
    t = batch * seq
    x = np.asarray(x, dtype=np.float32)
    xTb = np.ascontiguousarray(x.reshape(t, D).T).astype(bf)
    inv_freq = 1.0 / (
        ROPE_BASE ** (np.arange(0, HD, 2, dtype=np.float32) / HD)
    )
    pos = np.arange(seq, dtype=np.float32)
    freqs = np.einsum("i,j->ij", pos, inv_freq)
    emb = np.concatenate([freqs, freqs], axis=-1)            # [s, HD]
    cosT_np = np.ascontiguousarray(np.cos(emb).T).astype(np.float32)
    sinT = np.sin(emb).T.astype(np.float32)
    sinS_np = np.ascontiguousarray(
        np.concatenate([-sinT[: HD // 2], sinT[HD // 2 :]], axis=0)
    )
    in_maps = []
    for c in range(N_CORES):
        sl = slice(c * CH, (c + 1) * CH)
        in_maps.append(
            {
                "xT": xTb,
                "wqT": np.ascontiguousarray(np.asarray(Wq)[sl].T).astype(bf),
                "wkT": np.ascontiguousarray(np.asarray(Wk)[sl].T).astype(bf),
                "wvT": np.ascontiguousarray(np.asarray(Wv)[sl].T).astype(bf),
                "woT": np.ascontiguousarray(np.asarray(Wo)[sl].T).astype(bf),
                "cosT": cosT_np,
                "sinS": sinS_np,
            }
        )
    return in_maps


_PROGRAM = None


def kernel(x, Wq, Wk, Wv, Wo):
    global _PROGRAM
    if _PROGRAM is None:
        _PROGRAM = build_program()
    in_maps = host_inputs(x, Wq, Wk, Wv, Wo)
    res = run_bass_kernel_spmd(_PROGRAM, in_maps, list(range(N_CORES)))
    outT_full = np.concatenate(
        [res.results[c]["outT"] for c in range(N_CORES)], axis=0
    )  # [D, T]
    return np.ascontiguousarray(outT_full.T).reshape(B, S, D).astype(np.float32)


if __name__ == "__main__":
    xs = np.random.randn(B, S, D).astype(np.float32)
    ws = [
        (np.random.randn(D, D) * D**-0.5).astype(np.float32) for _ in range(4)
    ]
    out = kernel(xs, *ws)
    print(out.shape, out.dtype)
